# revision 37
# baseline (speedup 1.0000x reference)
"""Trainium2 Bass kernel for nn_BinaryMemory (retrieval_knn).

reference:
    gated = sigmoid(query @ W.T + b)                      # [1, D], D=4096
    sims  = 1 - mean(|memory - gated|, axis=-1)           # [N],   N=16384
    mask  = sims >= 0.8

Sharding (8 cores, no collectives): shard the D axis. Core c owns
d-chunk [c*512, (c+1)*512):
  - W rows c*512..c*512+511  -> computes gated[c*512:(c+1)*512] locally
    (dot products via scalar_tensor_tensor with sum-accumulate on DVE)
  - memory[:, c*512:(c+1)*512] -> partial L1 sums over its d-chunk for
    all 16384 rows
  - outputs partial sums [128, 128]; host reindexes, sums the 8 cores'
    partials and applies sims = 1 - s/D, mask = sims >= 0.8.

Per-tile pipeline: DVE subtract + 8x ScalarE Abs-with-accumulate; the
last two tiles use the DVE fused abs-reduce so ScalarE is off the tail.
The gated row vector is broadcast to all partitions with PE row-select
matmuls (no DMA on the gate critical path); query arrives already
broadcast from the host (a step-0 broadcast DMA re-reading one DRAM
region 128x is HBM-bank-conflict bound: measured ~36 us for 2 MB).
GpSimd only triggers DMAs (its tensor ops contend with DVE for SBUF
ports; measured both slow ~1.5x when concurrent).

Memory tile t holds rows t*1024..t*1024+1023; partition p holds the 8
consecutive rows t*1024+p*8+j (16 KB contiguous DMA runs). Per-core HBM
traffic ~42 MB (memory regime).
"""
import sys

sys.path.insert(0, "/opt/trn_rl_repo")

import numpy as np

import concourse.bacc as bacc
import concourse.mybir as mybir
import concourse.tile as tile
from concourse.bass_utils import run_bass_kernel_spmd
from concourse.tile_rust import add_dep_helper

N_CORES = 8
D = 4096
N = 16384
D_SH = D // N_CORES          # 512 dims per core
W_TILES = D_SH // 128        # 4 gate-weight tiles [128, 4096]
GP = 8                       # row-groups packed per memory tile
M_TILES = N // (128 * GP)    # 16 memory tiles [128, 8*512]
THRESHOLD = 0.8

_CACHE = {}


def _build():
    f32 = mybir.dt.float32
    nc = bacc.Bacc(
        "TRN2", target_bir_lowering=False, debug=False, num_devices=N_CORES
    )

    qb = nc.dram_tensor("qb", [128, D], f32, kind="ExternalInput")
    w = nc.dram_tensor("w", [D_SH, D], f32, kind="ExternalInput")
    b = nc.dram_tensor("b", [D_SH], f32, kind="ExternalInput")
    mem = nc.dram_tensor("mem", [N, D_SH], f32, kind="ExternalInput")
    ident = nc.dram_tensor("ident", [128, 128], f32, kind="ExternalInput")
    # sel[k, wt*128+m] = 1 iff k==wt (row-select stationaries)
    sel = nc.dram_tensor(
        "sel", [W_TILES, W_TILES * 128], f32, kind="ExternalInput"
    )
    partials = nc.dram_tensor(
        "partials", [128, M_TILES * GP], f32, kind="ExternalOutput"
    )

    with tile.TileContext(nc) as tc:
        with (
            tc.tile_pool(name="const", bufs=1) as cpool,
            tc.tile_pool(name="big", bufs=7) as bpool,
            tc.tile_pool(name="diff", bufs=3) as dpool,
            tc.tile_pool(name="absout", bufs=2) as apool,
            tc.tile_pool(name="small", bufs=1) as spool,
            tc.tile_pool(name="psum", bufs=1, space="PSUM") as ppool,
        ):
            # The scalar-engine HWDGE ring carries ONLY gate traffic: a
            # gate-dependent DMA on the sync/gpsimd rings would block the
            # FIFO mem stream behind the gate.
            q_b = dpool.tile([128, D], f32, tag="diff")
            nc.scalar.dma_start(out=q_b[:], in_=qb[:])
            id_sb = cpool.tile([128, 128], f32, tag="ident")
            nc.scalar.dma_start(out=id_sb[:], in_=ident[:])
            sel_sb = spool.tile([W_TILES, W_TILES * 128], f32, tag="sel")
            nc.scalar.dma_start(out=sel_sb[:], in_=sel[:])
            b_row = spool.tile([W_TILES, 128], f32, tag="brow")
            nc.scalar.dma_start(
                out=b_row[:], in_=b[:].rearrange("(t p) -> t p", p=128)
            )

            # ---- gate: z[j] = sum_d W[j, d] * q[d], j = wt*128 + p ----
            z_col = spool.tile([128, W_TILES], f32, tag="zcol")
            w_dmas = []
            for wt in range(W_TILES):
                w_tile = bpool.tile([128, D], f32, tag="m")
                w_eng = nc.sync if wt % 2 == 0 else nc.gpsimd
                w_dmas.append(
                    w_eng.dma_start(
                        out=w_tile[:], in_=w[wt * 128 : (wt + 1) * 128, :]
                    )
                )
                scratch = dpool.tile([128, D], f32, tag="diff")
                nc.vector.scalar_tensor_tensor(
                    out=scratch[:],
                    in0=w_tile[:],
                    scalar=1.0,
                    in1=q_b[:],
                    op0=mybir.AluOpType.mult,
                    op1=mybir.AluOpType.mult,
                    accum_out=z_col[:, wt : wt + 1],
                )

            # transpose z to row layout [wt, p]; add b, sigmoid there.
            # The little z transpose parks in a corner of the g PSUM tile
            # (Tile orders the later overwrite after the reads).
            g_ps = ppool.tile([128, D_SH], f32, tag="gps")
            z_ps = g_ps[0:W_TILES, 0:128]
            nc.tensor.transpose(z_ps, z_col[:], id_sb[:])
            g_row = spool.tile([W_TILES, 128], f32, tag="grow")
            nc.vector.tensor_add(g_row[:], z_ps, b_row[:])
            nc.scalar.activation(
                g_row[:], g_row[:], mybir.ActivationFunctionType.Sigmoid
            )
            # broadcast g straight from g_row [4,128]: matmul with the
            # row-select stationary sel_wt gives out[p, n] = g_row[wt, n]
            # for every partition p -- no DMA in the chain.
            for wt in range(W_TILES):
                nc.tensor.matmul(
                    g_ps[:, wt * 128 : (wt + 1) * 128],
                    sel_sb[:, wt * 128 : (wt + 1) * 128],
                    g_row[:],
                )
            # materialize the replicated gate row: plain 2D APs measure
            # faster than step-0 broadcast APs on the hot subtract
            g_rep = cpool.tile([128, GP * D_SH], f32, tag="grep")
            nc.vector.tensor_copy(g_rep[:, 0:D_SH], g_ps[:])
            for j in range(1, GP):
                nc.vector.tensor_copy(
                    g_rep[:, j * D_SH : (j + 1) * D_SH], g_rep[:, 0:D_SH]
                )

            # ---- sims partials ----
            # tile t: partition p, free (j, d) = mem[t*1024 + p*8 + j, d]
            memv = mem[:].rearrange("(t p j) d -> t p j d", p=128, j=GP)
            sums = spool.tile([128, M_TILES * GP], f32, tag="sums")
            for t in range(M_TILES):
                m_tile = bpool.tile([128, GP * D_SH], f32, tag="m")
                dma_eng = nc.sync if t % 2 == 0 else nc.gpsimd
                m_dma = dma_eng.dma_start(
                    out=m_tile[:].rearrange("p (j d) -> p j d", j=GP),
                    in_=memv[t],
                )
                if t < 2:
                    # hold the mem stream until the gate weights are in:
                    # concurrent mem DMAs dilute W's share of HBM BW and
                    # delay the whole compute stream
                    for wd in w_dmas:
                        add_dep_helper(
                            m_dma.ins,
                            wd.ins,
                            sync=True,
                            reason="mem stream after gate weights",
                        )
                diff = dpool.tile([128, GP * D_SH], f32, tag="diff")
                nc.vector.tensor_sub(diff[:], m_tile[:], g_rep[:])
                if t in (5, 9, 13):
                    # DVE fused abs-reduce on the last tiles: shorter
                    # drain, keeps ScalarE off the critical tail
                    nc.vector.tensor_reduce(
                        out=sums[:, t * GP : (t + 1) * GP],
                        in_=diff[:].rearrange("p (j d) -> p j d", j=GP),
                        axis=mybir.AxisListType.X,
                        op=mybir.AluOpType.add,
                        apply_absolute_value=True,
                    )
                else:
                    for j in range(GP):
                        a_out = apool.tile([128, D_SH], f32, tag="absout")
                        col = t * GP + j
                        nc.scalar.activation(
                            a_out[:],
                            diff[:, j * D_SH : (j + 1) * D_SH],
                            mybir.ActivationFunctionType.Abs,
                            accum_out=sums[:, col : col + 1],
                        )

            nc.sync.dma_start(out=partials[:], in_=sums[:])

    nc.compile()
    return nc


def _get_nc():
    if "nc" not in _CACHE:
        _CACHE["nc"] = _build()
    return _CACHE["nc"]


def make_aux_inputs():
    ident = np.eye(128, dtype=np.float32)
    sel = np.zeros((W_TILES, W_TILES * 128), dtype=np.float32)
    for wt in range(W_TILES):
        sel[wt, wt * 128 : (wt + 1) * 128] = 1.0
    return ident, sel


def kernel(query, W, b, memory, _trace=False, _return_raw=False):
    query = np.asarray(query, dtype=np.float32)
    W = np.asarray(W, dtype=np.float32)
    b = np.asarray(b, dtype=np.float32)
    memory = np.asarray(memory, dtype=np.float32)
    ident, sel = make_aux_inputs()
    q_bcast = np.ascontiguousarray(np.broadcast_to(query.reshape(1, D), (128, D)))

    in_maps = []
    for c in range(N_CORES):
        sl = slice(c * D_SH, (c + 1) * D_SH)
        in_maps.append(
            {
                "qb": q_bcast,
                "w": np.ascontiguousarray(W[sl, :]),
                "b": np.ascontiguousarray(b[sl]),
                "mem": np.ascontiguousarray(memory[:, sl]),
                "ident": ident,
                "sel": sel,
            }
        )

    nc = _get_nc()
    res = run_bass_kernel_spmd(
        nc, in_maps, list(range(N_CORES)), trace=_trace
    )

    total = np.zeros(N, dtype=np.float64)
    for c in range(N_CORES):
        mat = res.results[c]["partials"]  # [128 (p), 128 (t*8+j)]
        # row n = t*1024 + p*8 + j
        part = mat.reshape(128, M_TILES, GP).transpose(1, 0, 2).reshape(N)
        total += part.astype(np.float64)
    sims = (1.0 - total / D).astype(np.float32)
    mask = sims >= THRESHOLD
    if _return_raw:
        return (sims, mask), res
    return sims, mask


# revision 38
# speedup vs baseline: 1.6116x; 1.6116x over previous
"""Trainium2 Bass kernel for nn_BinaryMemory (retrieval_knn).

reference:
    gated = sigmoid(query @ W.T + b)                      # [1, D], D=4096
    sims  = 1 - mean(|memory - gated|, axis=-1)           # [N],   N=16384
    mask  = sims >= 0.8

Sharding (8 cores, no collectives): shard the D axis. Core c owns
d-chunk [c*512, (c+1)*512):
  - W rows c*512..c*512+511  -> computes gated[c*512:(c+1)*512] locally
    (dot products via scalar_tensor_tensor with f32 sum-accumulate on DVE)
  - memory[:, c*512:(c+1)*512] -> partial L1 sums over its d-chunk for
    all 16384 rows
  - outputs partial sums [128, 128] f32; host reindexes, sums the 8
    cores' partials and applies sims = 1 - s/D, mask = sims >= 0.8.

W / query / memory stream in as fp16 (host-side cast): halves the HBM
traffic of this memory-bound kernel and gives the DVE 16-bit 2x mode on
the hot subtract. All reductions accumulate in f32; quantization error
on sims is ~5e-6 relative (f32 build measures 1.7e-7).

Per-tile pipeline: DVE subtract + 8x ScalarE Abs-with-accumulate, with
6 of 16 tiles handled entirely on DVE via the fused abs-reduce so the
two engines finish together. The gated row is broadcast to partitions
with PE row-select matmuls (no DMA on the gate critical path). GpSimd
only triggers DMAs (its tensor ops contend with DVE for SBUF ports).

Memory tile t holds rows t*1024..t*1024+1023; partition p holds the 8
consecutive rows t*1024+p*8+j (8 KB contiguous DMA runs). Per-core HBM
traffic ~21 MB.
"""
import sys

sys.path.insert(0, "/opt/trn_rl_repo")

import numpy as np

import concourse.bacc as bacc
import concourse.mybir as mybir
import concourse.tile as tile
from concourse.bass_utils import run_bass_kernel_spmd

N_CORES = 8
D = 4096
N = 16384
D_SH = D // N_CORES          # 512 dims per core
W_TILES = D_SH // 128        # 4 gate-weight tiles [128, 4096]
GP = 8                       # row-groups packed per memory tile
M_TILES = N // (128 * GP)    # 16 memory tiles [128, 8*512]
THRESHOLD = 0.8
A_TILES = {2, 5, 8, 11, 13, 15}   # DVE-only abs-reduce tiles

_CACHE = {}


def _build():
    f32 = mybir.dt.float32
    f16 = mybir.dt.float16
    nc = bacc.Bacc(
        "TRN2", target_bir_lowering=False, debug=False, num_devices=N_CORES
    )

    qb = nc.dram_tensor("qb", [128, D], f16, kind="ExternalInput")
    w = nc.dram_tensor("w", [D_SH, D], f16, kind="ExternalInput")
    b = nc.dram_tensor("b", [D_SH], f32, kind="ExternalInput")
    mem = nc.dram_tensor("mem", [N, D_SH], f16, kind="ExternalInput")
    ident = nc.dram_tensor("ident", [128, 128], f32, kind="ExternalInput")
    # sel[k, wt*128+m] = 1 iff k==wt (row-select stationaries)
    sel = nc.dram_tensor(
        "sel", [W_TILES, W_TILES * 128], f16, kind="ExternalInput"
    )
    partials = nc.dram_tensor(
        "partials", [128, M_TILES * GP], f32, kind="ExternalOutput"
    )

    with tile.TileContext(nc) as tc:
        with (
            tc.tile_pool(name="const", bufs=1) as cpool,
            tc.tile_pool(name="big", bufs=9) as bpool,
            tc.tile_pool(name="diff", bufs=4) as dpool,
            tc.tile_pool(name="absout", bufs=2) as apool,
            tc.tile_pool(name="small", bufs=1) as spool,
            tc.tile_pool(name="psum", bufs=1, space="PSUM") as ppool,
        ):
            # The scalar-engine HWDGE ring carries ONLY gate traffic: a
            # gate-dependent DMA on the sync/gpsimd rings would block the
            # FIFO mem stream behind the gate.
            q_b = dpool.tile([128, D], f16, tag="diff")
            nc.scalar.dma_start(out=q_b[:], in_=qb[:])
            id_sb = cpool.tile([128, 128], f32, tag="ident")
            nc.scalar.dma_start(out=id_sb[:], in_=ident[:])
            sel_sb = spool.tile([W_TILES, W_TILES * 128], f16, tag="sel")
            nc.scalar.dma_start(out=sel_sb[:], in_=sel[:])
            b_row = spool.tile([W_TILES, 128], f32, tag="brow")
            nc.scalar.dma_start(
                out=b_row[:], in_=b[:].rearrange("(t p) -> t p", p=128)
            )

            # ---- gate: z[j] = sum_d W[j, d] * q[d], j = wt*128 + p ----
            z_col = spool.tile([128, W_TILES], f32, tag="zcol")
            for wt in range(W_TILES):
                w_tile = bpool.tile([128, D], f16, tag="m")
                w_eng = nc.sync if wt % 2 == 0 else nc.gpsimd
                w_eng.dma_start(
                    out=w_tile[:], in_=w[wt * 128 : (wt + 1) * 128, :]
                )
                scratch = dpool.tile([128, D], f16, tag="diff")
                nc.vector.scalar_tensor_tensor(
                    out=scratch[:],
                    in0=w_tile[:],
                    scalar=1.0,
                    in1=q_b[:],
                    op0=mybir.AluOpType.mult,
                    op1=mybir.AluOpType.mult,
                    accum_out=z_col[:, wt : wt + 1],
                )

            # transpose z to row layout [wt, p]; add b, sigmoid there.
            # The little z transpose parks in a corner of the g PSUM tile
            # (Tile orders the later overwrite after the reads).
            g_ps = ppool.tile([128, D_SH], f32, tag="gps")
            z_ps = g_ps[0:W_TILES, 0:128]
            nc.tensor.transpose(z_ps, z_col[:], id_sb[:])
            zb_row = spool.tile([W_TILES, 128], f32, tag="zbrow")
            nc.vector.tensor_add(zb_row[:], z_ps, b_row[:])
            g_row = spool.tile([W_TILES, 128], f16, tag="grow")
            nc.scalar.activation(
                g_row[:], zb_row[:], mybir.ActivationFunctionType.Sigmoid
            )
            # broadcast g straight from g_row [4,128]: matmul with the
            # row-select stationary sel_wt gives out[p, n] = g_row[wt, n]
            # for every partition p -- no DMA in the chain.
            for wt in range(W_TILES):
                nc.tensor.matmul(
                    g_ps[:, wt * 128 : (wt + 1) * 128],
                    sel_sb[:, wt * 128 : (wt + 1) * 128],
                    g_row[:],
                )
            # materialize the replicated gate row in fp16 (plain 2D APs
            # measure faster than step-0 broadcast APs on the hot subtract)
            g_rep = cpool.tile([128, GP * D_SH], f16, tag="grep")
            nc.vector.tensor_copy(g_rep[:, 0:D_SH], g_ps[:])
            for j in range(1, GP):
                nc.vector.tensor_copy(
                    g_rep[:, j * D_SH : (j + 1) * D_SH], g_rep[:, 0:D_SH]
                )

            # ---- sims partials ----
            # tile t: partition p, free (j, d) = mem[t*1024 + p*8 + j, d]
            memv = mem[:].rearrange("(t p j) d -> t p j d", p=128, j=GP)
            sums = spool.tile([128, M_TILES * GP], f32, tag="sums")
            for t in range(M_TILES):
                m_tile = bpool.tile([128, GP * D_SH], f16, tag="m")
                dma_eng = nc.sync if t % 2 == 0 else nc.gpsimd
                dma_eng.dma_start(
                    out=m_tile[:].rearrange("p (j d) -> p j d", j=GP),
                    in_=memv[t],
                )
                diff = dpool.tile([128, GP * D_SH], f16, tag="diff")
                nc.vector.tensor_sub(diff[:], m_tile[:], g_rep[:])
                if t in A_TILES:
                    nc.vector.tensor_reduce(
                        out=sums[:, t * GP : (t + 1) * GP],
                        in_=diff[:].rearrange("p (j d) -> p j d", j=GP),
                        axis=mybir.AxisListType.X,
                        op=mybir.AluOpType.add,
                        apply_absolute_value=True,
                    )
                else:
                    for j in range(GP):
                        a_out = apool.tile([128, D_SH], f16, tag="absout")
                        col = t * GP + j
                        nc.scalar.activation(
                            a_out[:],
                            diff[:, j * D_SH : (j + 1) * D_SH],
                            mybir.ActivationFunctionType.Abs,
                            accum_out=sums[:, col : col + 1],
                        )

            nc.sync.dma_start(out=partials[:], in_=sums[:])

    nc.compile()
    return nc


def _get_nc():
    if "nc" not in _CACHE:
        _CACHE["nc"] = _build()
    return _CACHE["nc"]


def make_aux_inputs():
    ident = np.eye(128, dtype=np.float32)
    sel = np.zeros((W_TILES, W_TILES * 128), dtype=np.float16)
    for wt in range(W_TILES):
        sel[wt, wt * 128 : (wt + 1) * 128] = 1.0
    return ident, sel


def kernel(query, W, b, memory, _trace=False, _return_raw=False):
    query = np.asarray(query, dtype=np.float32)
    W = np.asarray(W, dtype=np.float32)
    b = np.asarray(b, dtype=np.float32)
    memory = np.asarray(memory, dtype=np.float32)
    ident, sel = make_aux_inputs()
    q_bcast = np.ascontiguousarray(
        np.broadcast_to(query.reshape(1, D).astype(np.float16), (128, D))
    )
    W16 = W.astype(np.float16)
    mem16 = memory.astype(np.float16)

    in_maps = []
    for c in range(N_CORES):
        sl = slice(c * D_SH, (c + 1) * D_SH)
        in_maps.append(
            {
                "qb": q_bcast,
                "w": np.ascontiguousarray(W16[sl, :]),
                "b": np.ascontiguousarray(b[sl]),
                "mem": np.ascontiguousarray(mem16[:, sl]),
                "ident": ident,
                "sel": sel,
            }
        )

    nc = _get_nc()
    res = run_bass_kernel_spmd(
        nc, in_maps, list(range(N_CORES)), trace=_trace
    )

    total = np.zeros(N, dtype=np.float64)
    for c in range(N_CORES):
        mat = res.results[c]["partials"]  # [128 (p), 128 (t*8+j)]
        # row n = t*1024 + p*8 + j
        part = mat.reshape(128, M_TILES, GP).transpose(1, 0, 2).reshape(N)
        total += part.astype(np.float64)
    sims = (1.0 - total / D).astype(np.float32)
    mask = sims >= THRESHOLD
    if _return_raw:
        return (sims, mask), res
    return sims, mask


# revision 39
# speedup vs baseline: 1.6287x; 1.0106x over previous
"""Trainium2 Bass kernel for nn_BinaryMemory (retrieval_knn).

reference:
    gated = sigmoid(query @ W.T + b)                      # [1, D], D=4096
    sims  = 1 - mean(|memory - gated|, axis=-1)           # [N],   N=16384
    mask  = sims >= 0.8

Sharding (8 cores, no collectives): shard the D axis. Core c owns
d-chunk [c*512, (c+1)*512):
  - W rows c*512..c*512+511  -> computes gated[c*512:(c+1)*512] locally
    (dot products via scalar_tensor_tensor with f32 sum-accumulate on DVE)
  - memory[:, c*512:(c+1)*512] -> partial L1 sums over its d-chunk for
    all 16384 rows
  - outputs partial sums [128, 128] f32; host reindexes, sums the 8
    cores' partials and applies sims = 1 - s/D, mask = sims >= 0.8.

W / query / memory stream in as fp16 (host-side cast): halves the HBM
traffic of this memory-bound kernel and gives the DVE 16-bit 2x mode on
the hot subtract. All reductions accumulate in f32; quantization error
on sims is ~5e-6 relative (f32 build measures 1.7e-7).

Per-tile pipeline: DVE subtract + 8x ScalarE Abs-with-accumulate, with
6 of 16 tiles handled entirely on DVE via the fused abs-reduce so the
two engines finish together. The gated row is broadcast to partitions
with PE row-select matmuls (no DMA on the gate critical path). All bulk
DMAs ride the sync HWDGE ring: one ring sustains ~309 GB/s (vs 247 mixed
with SWDGE) and its per-engine FIFO guarantees the gate weights land
before the mem stream starts competing for HBM.

Memory tile t holds rows t*1024..t*1024+1023; partition p holds the 8
consecutive rows t*1024+p*8+j (8 KB contiguous DMA runs). Per-core HBM
traffic ~21 MB.
"""
import sys

sys.path.insert(0, "/opt/trn_rl_repo")

import numpy as np

import concourse.bacc as bacc
import concourse.mybir as mybir
import concourse.tile as tile
from concourse.bass_utils import run_bass_kernel_spmd

N_CORES = 8
D = 4096
N = 16384
D_SH = D // N_CORES          # 512 dims per core
W_TILES = D_SH // 128        # 4 gate-weight tiles [128, 4096]
GP = 8                       # row-groups packed per memory tile
M_TILES = N // (128 * GP)    # 16 memory tiles [128, 8*512]
THRESHOLD = 0.8
A_TILES = {2, 5, 8, 11, 13, 15}   # DVE-only abs-reduce tiles

_CACHE = {}


def _build():
    f32 = mybir.dt.float32
    f16 = mybir.dt.float16
    nc = bacc.Bacc(
        "TRN2", target_bir_lowering=False, debug=False, num_devices=N_CORES
    )

    qb = nc.dram_tensor("qb", [128, D], f16, kind="ExternalInput")
    w = nc.dram_tensor("w", [D_SH, D], f16, kind="ExternalInput")
    b = nc.dram_tensor("b", [D_SH], f32, kind="ExternalInput")
    mem = nc.dram_tensor("mem", [N, D_SH], f16, kind="ExternalInput")
    ident = nc.dram_tensor("ident", [128, 128], f32, kind="ExternalInput")
    # sel[k, wt*128+m] = 1 iff k==wt (row-select stationaries)
    sel = nc.dram_tensor(
        "sel", [W_TILES, W_TILES * 128], f16, kind="ExternalInput"
    )
    partials = nc.dram_tensor(
        "partials", [128, M_TILES * GP], f32, kind="ExternalOutput"
    )

    with tile.TileContext(nc) as tc:
        with (
            tc.tile_pool(name="const", bufs=1) as cpool,
            tc.tile_pool(name="big", bufs=9) as bpool,
            tc.tile_pool(name="diff", bufs=4) as dpool,
            tc.tile_pool(name="absout", bufs=2) as apool,
            tc.tile_pool(name="small", bufs=1) as spool,
            tc.tile_pool(name="psum", bufs=1, space="PSUM") as ppool,
        ):
            # The scalar-engine HWDGE ring carries ONLY gate traffic: a
            # gate-dependent DMA on the sync/gpsimd rings would block the
            # FIFO mem stream behind the gate.
            q_b = dpool.tile([128, D], f16, tag="diff")
            nc.scalar.dma_start(out=q_b[:], in_=qb[:])
            id_sb = cpool.tile([128, 128], f32, tag="ident")
            nc.scalar.dma_start(out=id_sb[:], in_=ident[:])
            sel_sb = spool.tile([W_TILES, W_TILES * 128], f16, tag="sel")
            nc.scalar.dma_start(out=sel_sb[:], in_=sel[:])
            b_row = spool.tile([W_TILES, 128], f32, tag="brow")
            nc.scalar.dma_start(
                out=b_row[:], in_=b[:].rearrange("(t p) -> t p", p=128)
            )

            # ---- gate: z[j] = sum_d W[j, d] * q[d], j = wt*128 + p ----
            z_col = spool.tile([128, W_TILES], f32, tag="zcol")
            for wt in range(W_TILES):
                w_tile = bpool.tile([128, D], f16, tag="m")
                nc.sync.dma_start(
                    out=w_tile[:], in_=w[wt * 128 : (wt + 1) * 128, :]
                )
                scratch = dpool.tile([128, D], f16, tag="diff")
                nc.vector.scalar_tensor_tensor(
                    out=scratch[:],
                    in0=w_tile[:],
                    scalar=1.0,
                    in1=q_b[:],
                    op0=mybir.AluOpType.mult,
                    op1=mybir.AluOpType.mult,
                    accum_out=z_col[:, wt : wt + 1],
                )

            # transpose z to row layout [wt, p]; add b, sigmoid there.
            # The little z transpose parks in a corner of the g PSUM tile
            # (Tile orders the later overwrite after the reads).
            g_ps = ppool.tile([128, D_SH], f32, tag="gps")
            z_ps = g_ps[0:W_TILES, 0:128]
            nc.tensor.transpose(z_ps, z_col[:], id_sb[:])
            zb_row = spool.tile([W_TILES, 128], f32, tag="zbrow")
            nc.vector.tensor_add(zb_row[:], z_ps, b_row[:])
            g_row = spool.tile([W_TILES, 128], f16, tag="grow")
            nc.scalar.activation(
                g_row[:], zb_row[:], mybir.ActivationFunctionType.Sigmoid
            )
            # broadcast g straight from g_row [4,128]: matmul with the
            # row-select stationary sel_wt gives out[p, n] = g_row[wt, n]
            # for every partition p -- no DMA in the chain.
            for wt in range(W_TILES):
                nc.tensor.matmul(
                    g_ps[:, wt * 128 : (wt + 1) * 128],
                    sel_sb[:, wt * 128 : (wt + 1) * 128],
                    g_row[:],
                )
            # materialize the replicated gate row in fp16 (plain 2D APs
            # measure faster than step-0 broadcast APs on the hot subtract)
            g_rep = cpool.tile([128, GP * D_SH], f16, tag="grep")
            nc.vector.tensor_copy(g_rep[:, 0:D_SH], g_ps[:])
            for j in range(1, GP):
                nc.vector.tensor_copy(
                    g_rep[:, j * D_SH : (j + 1) * D_SH], g_rep[:, 0:D_SH]
                )

            # ---- sims partials ----
            # tile t: partition p, free (j, d) = mem[t*1024 + p*8 + j, d]
            memv = mem[:].rearrange("(t p j) d -> t p j d", p=128, j=GP)
            sums = spool.tile([128, M_TILES * GP], f32, tag="sums")
            for t in range(M_TILES):
                m_tile = bpool.tile([128, GP * D_SH], f16, tag="m")
                nc.sync.dma_start(
                    out=m_tile[:].rearrange("p (j d) -> p j d", j=GP),
                    in_=memv[t],
                )
                diff = dpool.tile([128, GP * D_SH], f16, tag="diff")
                nc.vector.tensor_sub(diff[:], m_tile[:], g_rep[:])
                if t in A_TILES:
                    nc.vector.tensor_reduce(
                        out=sums[:, t * GP : (t + 1) * GP],
                        in_=diff[:].rearrange("p (j d) -> p j d", j=GP),
                        axis=mybir.AxisListType.X,
                        op=mybir.AluOpType.add,
                        apply_absolute_value=True,
                    )
                else:
                    for j in range(GP):
                        a_out = apool.tile([128, D_SH], f16, tag="absout")
                        col = t * GP + j
                        nc.scalar.activation(
                            a_out[:],
                            diff[:, j * D_SH : (j + 1) * D_SH],
                            mybir.ActivationFunctionType.Abs,
                            accum_out=sums[:, col : col + 1],
                        )

            nc.sync.dma_start(out=partials[:], in_=sums[:])

    nc.compile()
    return nc


def _get_nc():
    if "nc" not in _CACHE:
        _CACHE["nc"] = _build()
    return _CACHE["nc"]


def make_aux_inputs():
    ident = np.eye(128, dtype=np.float32)
    sel = np.zeros((W_TILES, W_TILES * 128), dtype=np.float16)
    for wt in range(W_TILES):
        sel[wt, wt * 128 : (wt + 1) * 128] = 1.0
    return ident, sel


def kernel(query, W, b, memory, _trace=False, _return_raw=False):
    query = np.asarray(query, dtype=np.float32)
    W = np.asarray(W, dtype=np.float32)
    b = np.asarray(b, dtype=np.float32)
    memory = np.asarray(memory, dtype=np.float32)
    ident, sel = make_aux_inputs()
    q_bcast = np.ascontiguousarray(
        np.broadcast_to(query.reshape(1, D).astype(np.float16), (128, D))
    )
    W16 = W.astype(np.float16)
    mem16 = memory.astype(np.float16)

    in_maps = []
    for c in range(N_CORES):
        sl = slice(c * D_SH, (c + 1) * D_SH)
        in_maps.append(
            {
                "qb": q_bcast,
                "w": np.ascontiguousarray(W16[sl, :]),
                "b": np.ascontiguousarray(b[sl]),
                "mem": np.ascontiguousarray(mem16[:, sl]),
                "ident": ident,
                "sel": sel,
            }
        )

    nc = _get_nc()
    res = run_bass_kernel_spmd(
        nc, in_maps, list(range(N_CORES)), trace=_trace
    )

    total = np.zeros(N, dtype=np.float64)
    for c in range(N_CORES):
        mat = res.results[c]["partials"]  # [128 (p), 128 (t*8+j)]
        # row n = t*1024 + p*8 + j
        part = mat.reshape(128, M_TILES, GP).transpose(1, 0, 2).reshape(N)
        total += part.astype(np.float64)
    sims = (1.0 - total / D).astype(np.float32)
    mask = sims >= THRESHOLD
    if _return_raw:
        return (sims, mask), res
    return sims, mask


# revision 40
# speedup vs baseline: 1.6391x; 1.0064x over previous
"""Trainium2 Bass kernel for nn_BinaryMemory (retrieval_knn).

reference:
    gated = sigmoid(query @ W.T + b)                      # [1, D], D=4096
    sims  = 1 - mean(|memory - gated|, axis=-1)           # [N],   N=16384
    mask  = sims >= 0.8

Sharding (8 cores, no collectives): shard the D axis. Core c owns
d-chunk [c*512, (c+1)*512):
  - W rows c*512..c*512+511  -> computes gated[c*512:(c+1)*512] locally
    (dot products via scalar_tensor_tensor with f32 sum-accumulate on DVE)
  - memory[:, c*512:(c+1)*512] -> partial L1 sums over its d-chunk for
    all 16384 rows
  - outputs partial sums [128, 128] f32; host reindexes, sums the 8
    cores' partials and applies sims = 1 - s/D, mask = sims >= 0.8.

W / query / memory stream in as fp16 (host-side cast): halves the HBM
traffic of this memory-bound kernel and gives the DVE 16-bit 2x mode on
the hot subtract. All reductions accumulate in f32; quantization error
on sims is ~5e-6 relative (f32 build measures 1.7e-7).

Per-tile pipeline: DVE subtract + 8x ScalarE Abs-with-accumulate, with
6 of 16 tiles handled entirely on DVE via the fused abs-reduce so the
two engines finish together. The gated row is broadcast to partitions
with PE row-select matmuls (no DMA on the gate critical path). All bulk
DMAs ride the sync HWDGE ring: one ring sustains ~309 GB/s (vs 247 mixed
with SWDGE) and its per-engine FIFO guarantees the gate weights land
before the mem stream starts competing for HBM.

Memory tile t holds rows t*1024..t*1024+1023; partition p holds the 8
consecutive rows t*1024+p*8+j (8 KB contiguous DMA runs). Per-core HBM
traffic ~21 MB.
"""
import sys

sys.path.insert(0, "/opt/trn_rl_repo")

import numpy as np

import concourse.bacc as bacc
import concourse.mybir as mybir
import concourse.tile as tile
from concourse.bass_utils import run_bass_kernel_spmd

N_CORES = 8
D = 4096
N = 16384
D_SH = D // N_CORES          # 512 dims per core
W_TILES = D_SH // 128        # 4 gate-weight tiles [128, 4096]
GP = 8                       # row-groups packed per memory tile
M_TILES = N // (128 * GP)    # 16 memory tiles [128, 8*512]
THRESHOLD = 0.8
A_TILES = {2, 5, 8, 11, 13}       # DVE-only abs-reduce tiles
H_TILES = {14, 15}                # tail tiles: reduce+ABS split across engines

_CACHE = {}


def _build():
    f32 = mybir.dt.float32
    f16 = mybir.dt.float16
    nc = bacc.Bacc(
        "TRN2", target_bir_lowering=False, debug=False, num_devices=N_CORES
    )

    qb = nc.dram_tensor("qb", [128, D], f16, kind="ExternalInput")
    w = nc.dram_tensor("w", [D_SH, D], f16, kind="ExternalInput")
    b = nc.dram_tensor("b", [D_SH], f32, kind="ExternalInput")
    mem = nc.dram_tensor("mem", [N, D_SH], f16, kind="ExternalInput")
    ident = nc.dram_tensor("ident", [128, 128], f32, kind="ExternalInput")
    # sel[k, wt*128+m] = 1 iff k==wt (row-select stationaries)
    sel = nc.dram_tensor(
        "sel", [W_TILES, W_TILES * 128], f16, kind="ExternalInput"
    )
    partials = nc.dram_tensor(
        "partials", [128, M_TILES * GP], f32, kind="ExternalOutput"
    )

    with tile.TileContext(nc) as tc:
        with (
            tc.tile_pool(name="const", bufs=1) as cpool,
            tc.tile_pool(name="big", bufs=9) as bpool,
            tc.tile_pool(name="diff", bufs=4) as dpool,
            tc.tile_pool(name="absout", bufs=2) as apool,
            tc.tile_pool(name="small", bufs=1) as spool,
            tc.tile_pool(name="psum", bufs=1, space="PSUM") as ppool,
        ):
            # The scalar-engine HWDGE ring carries ONLY gate traffic: a
            # gate-dependent DMA on the sync/gpsimd rings would block the
            # FIFO mem stream behind the gate.
            q_b = dpool.tile([128, D], f16, tag="diff")
            nc.scalar.dma_start(out=q_b[:], in_=qb[:])
            id_sb = cpool.tile([128, 128], f32, tag="ident")
            nc.scalar.dma_start(out=id_sb[:], in_=ident[:])
            sel_sb = spool.tile([W_TILES, W_TILES * 128], f16, tag="sel")
            nc.scalar.dma_start(out=sel_sb[:], in_=sel[:])
            b_row = spool.tile([W_TILES, 128], f32, tag="brow")
            nc.scalar.dma_start(
                out=b_row[:], in_=b[:].rearrange("(t p) -> t p", p=128)
            )

            # ---- gate: z[j] = sum_d W[j, d] * q[d], j = wt*128 + p ----
            z_col = spool.tile([128, W_TILES], f32, tag="zcol")
            for wt in range(W_TILES):
                w_tile = bpool.tile([128, D], f16, tag="m")
                nc.sync.dma_start(
                    out=w_tile[:], in_=w[wt * 128 : (wt + 1) * 128, :]
                )
                # scalar_tensor_tensor has no 16-bit 2x uop (measured
                # 4.34 us); TT mult (2x, 2.2 us) + ScalarE Copy-accumulate
                # gets the dot product off the critical path sooner
                prod = dpool.tile([128, D], f16, tag="diff")
                nc.vector.tensor_mul(prod[:], w_tile[:], q_b[:])
                gacc = apool.tile([128, D], f16, tag="gacc")
                nc.scalar.activation(
                    gacc[:],
                    prod[:],
                    mybir.ActivationFunctionType.Copy,
                    accum_out=z_col[:, wt : wt + 1],
                )

            # transpose z to row layout [wt, p]; add b, sigmoid there.
            # The little z transpose parks in a corner of the g PSUM tile
            # (Tile orders the later overwrite after the reads).
            g_ps = ppool.tile([128, D_SH], f32, tag="gps")
            z_ps = g_ps[0:W_TILES, 0:128]
            nc.tensor.transpose(z_ps, z_col[:], id_sb[:])
            zb_row = spool.tile([W_TILES, 128], f32, tag="zbrow")
            nc.vector.tensor_add(zb_row[:], z_ps, b_row[:])
            g_row = spool.tile([W_TILES, 128], f16, tag="grow")
            nc.scalar.activation(
                g_row[:], zb_row[:], mybir.ActivationFunctionType.Sigmoid
            )
            # broadcast g straight from g_row [4,128]: matmul with the
            # row-select stationary sel_wt gives out[p, n] = g_row[wt, n]
            # for every partition p -- no DMA in the chain.
            for wt in range(W_TILES):
                nc.tensor.matmul(
                    g_ps[:, wt * 128 : (wt + 1) * 128],
                    sel_sb[:, wt * 128 : (wt + 1) * 128],
                    g_row[:],
                )
            # materialize the replicated gate row in fp16 (plain 2D APs
            # measure faster than step-0 broadcast APs on the hot subtract)
            g_rep = cpool.tile([128, GP * D_SH], f16, tag="grep")
            nc.vector.tensor_copy(g_rep[:, 0:D_SH], g_ps[:])
            for j in range(1, GP):
                nc.vector.tensor_copy(
                    g_rep[:, j * D_SH : (j + 1) * D_SH], g_rep[:, 0:D_SH]
                )

            # ---- sims partials ----
            # tile t: partition p, free (j, d) = mem[t*1024 + p*8 + j, d]
            memv = mem[:].rearrange("(t p j) d -> t p j d", p=128, j=GP)
            sums = spool.tile([128, M_TILES * GP], f32, tag="sums")
            for t in range(M_TILES):
                m_tile = bpool.tile([128, GP * D_SH], f16, tag="m")
                nc.sync.dma_start(
                    out=m_tile[:].rearrange("p (j d) -> p j d", j=GP),
                    in_=memv[t],
                )
                diff = dpool.tile([128, GP * D_SH], f16, tag="diff")
                nc.vector.tensor_sub(diff[:], m_tile[:], g_rep[:])
                if t in A_TILES:
                    nc.vector.tensor_reduce(
                        out=sums[:, t * GP : (t + 1) * GP],
                        in_=diff[:].rearrange("p (j d) -> p j d", j=GP),
                        axis=mybir.AxisListType.X,
                        op=mybir.AluOpType.add,
                        apply_absolute_value=True,
                    )
                elif t in H_TILES:
                    # tail: half the groups on each engine -> ~3 us drain
                    half = GP // 2
                    nc.vector.tensor_reduce(
                        out=sums[:, t * GP : t * GP + half],
                        in_=diff[:, 0 : half * D_SH].rearrange(
                            "p (j d) -> p j d", j=half
                        ),
                        axis=mybir.AxisListType.X,
                        op=mybir.AluOpType.add,
                        apply_absolute_value=True,
                    )
                    for j in range(half, GP):
                        a_out = apool.tile([128, D_SH], f16, tag="absout")
                        col = t * GP + j
                        nc.scalar.activation(
                            a_out[:],
                            diff[:, j * D_SH : (j + 1) * D_SH],
                            mybir.ActivationFunctionType.Abs,
                            accum_out=sums[:, col : col + 1],
                        )
                else:
                    for j in range(GP):
                        a_out = apool.tile([128, D_SH], f16, tag="absout")
                        col = t * GP + j
                        nc.scalar.activation(
                            a_out[:],
                            diff[:, j * D_SH : (j + 1) * D_SH],
                            mybir.ActivationFunctionType.Abs,
                            accum_out=sums[:, col : col + 1],
                        )

            nc.sync.dma_start(out=partials[:], in_=sums[:])

    nc.compile()
    return nc


def _get_nc():
    if "nc" not in _CACHE:
        _CACHE["nc"] = _build()
    return _CACHE["nc"]


def make_aux_inputs():
    ident = np.eye(128, dtype=np.float32)
    sel = np.zeros((W_TILES, W_TILES * 128), dtype=np.float16)
    for wt in range(W_TILES):
        sel[wt, wt * 128 : (wt + 1) * 128] = 1.0
    return ident, sel


def kernel(query, W, b, memory, _trace=False, _return_raw=False):
    query = np.asarray(query, dtype=np.float32)
    W = np.asarray(W, dtype=np.float32)
    b = np.asarray(b, dtype=np.float32)
    memory = np.asarray(memory, dtype=np.float32)
    ident, sel = make_aux_inputs()
    q_bcast = np.ascontiguousarray(
        np.broadcast_to(query.reshape(1, D).astype(np.float16), (128, D))
    )
    W16 = W.astype(np.float16)
    mem16 = memory.astype(np.float16)

    in_maps = []
    for c in range(N_CORES):
        sl = slice(c * D_SH, (c + 1) * D_SH)
        in_maps.append(
            {
                "qb": q_bcast,
                "w": np.ascontiguousarray(W16[sl, :]),
                "b": np.ascontiguousarray(b[sl]),
                "mem": np.ascontiguousarray(mem16[:, sl]),
                "ident": ident,
                "sel": sel,
            }
        )

    nc = _get_nc()
    res = run_bass_kernel_spmd(
        nc, in_maps, list(range(N_CORES)), trace=_trace
    )

    total = np.zeros(N, dtype=np.float64)
    for c in range(N_CORES):
        mat = res.results[c]["partials"]  # [128 (p), 128 (t*8+j)]
        # row n = t*1024 + p*8 + j
        part = mat.reshape(128, M_TILES, GP).transpose(1, 0, 2).reshape(N)
        total += part.astype(np.float64)
    sims = (1.0 - total / D).astype(np.float32)
    mask = sims >= THRESHOLD
    if _return_raw:
        return (sims, mask), res
    return sims, mask
